# revision 4
# baseline (speedup 1.0000x reference)
"""CASSI colored-aperture layer (nn_CASSI_layer_Colored) on 8 Trainium2 NeuronCores.

Reference semantics (B=4, M=N=KERN=256, L=24 bands, S=22 shots):
    H[m,n,l,s] = (wr*fr[l] + wg*fg[l] + wb*fb[l] + wc*fc[l]) / (wr+wg+wb+wc)
    Y[b,m,n',s] = sum_l H[m,n'-l,l,s] * x[b,m,n'-l,l]          (dispersion shift-sum)
    X[b,m,n,l]  = sum_s H[m,n,l,s] * Y[b,m,n+l,s]              (adjoint + shot sum)
    out = X / max(X)

Sharding: data-parallel over (batch b, row-half mh): 4 x 2 = 8 cores.  Rows m
never couple, so each core computes 128 rows of one batch independently; only
the final global max couples shards (host side, after the gather).

Per-core mapping: partitions = 128 m-rows; free dims are s-major (s, n) so the
dispersion shift n -> n+l is a free-dim offset, the broadcast of x over s is a
stride-0 outer AP dim (dense innermost keeps DVE 2x mode), and the shot-sum
becomes contiguous stripe-halving adds.  Pipeline is fp16 (~1e-3 max rel err
vs fp64, validated).  Per band l:
  stage 1: h_l = sum_c F[c,l]*a_c (ScalarE seeds + partials, DVE/GpSimd adds),
           Y[:, l:l+N] += h_l * x[:, l-bcast]: on 2/3 of bands the add runs as
           a DMA-engine CCE accumulate (dst AP must stay strided: contiguous
           dst APs silently drop the accumulate), relieving GpSimd/DVE; the
           rest keep the GpSimd/DVE stripe split.  h_l spilled to DRAM.
  stage 2: h_l reloaded (DMA, hidden), t = h_l * Y[:, l:l+N] (DVE),
           X[:, l] = stripe-tree shot sum (GpSimd first level, DVE rest)
"""

import numpy as np

B, M, N, L, S = 4, 256, 256, 24, 22
MSH = M // 2                     # rows per core
NCORES = 8
NS, NL = N * S, N * L
NP = N + L - 1                   # 279 shifted columns
YW = NP * S                      # Y free width (s-major: s outer, n' inner)


def _bases() -> np.ndarray:
    """(4, L) color responses paired row-wise with (wr, wg, wb, wc)."""
    wl = np.linspace(400.0, 700.0, L)

    def g(mu: float, sig: float) -> np.ndarray:
        return np.exp(-0.5 * ((wl - mu) / sig) ** 2)

    # reference: H = wr*f620 + wg*f550 + wb*f450 + wc*f500 (fr,fg,fc,fb = 620,550,500,450)
    return np.stack([g(620.0, 50.0), g(550.0, 50.0), g(450.0, 50.0), g(500.0, 50.0)])


_NC = None


def _build():
    import concourse.bacc as bacc
    import concourse.mybir as mybir
    import concourse.tile as tile

    f16, f32 = mybir.dt.float16, mybir.dt.float32
    A = mybir.AluOpType
    F = _bases()

    nc = bacc.Bacc("TRN2", target_bir_lowering=False, debug=False, num_devices=NCORES)
    xin = nc.declare_dram_parameter("x16", [MSH, NL], f16, isOutput=False)   # (l, n)
    wins = [
        nc.declare_dram_parameter(f"w{i}", [MSH, NS], f16, isOutput=False)   # (s, n)
        for i in range(4)
    ]
    out = nc.declare_dram_parameter("out", [MSH, NL], f32, isOutput=True)    # (l, n)
    hcache = nc.dram_tensor("hcache", [L, MSH, NS], f16)

    with tile.TileContext(nc) as tc:
        with (
            tc.tile_pool(name="main", bufs=1) as main,
            tc.tile_pool(name="hp", bufs=3) as hp,
            tc.tile_pool(name="tp", bufs=4) as tp,
            tc.tile_pool(name="pp", bufs=2) as pp,
        ):
            a = [main.tile([MSH, NS], f16, tag=f"a{i}", name=f"a{i}") for i in range(4)]
            xt = main.tile([MSH, NL], f16, tag="x", bufs=2, name="xt")
            Y = main.tile([MSH, YW], f16, tag="Y", name="Yt")

            for i in range(4):
                nc.sync.dma_start(a[i][:], wins[i][:])
            nc.sync.dma_start(xt[:], xin[:])
            nc.gpsimd.memset(Y[:], 0.0)

            # a_c = w_c / (wr+wg+wb+wc)
            u = hp.tile([MSH, NS], f16, tag="h", name="ut")
            nc.vector.tensor_tensor(u[:], a[0][:], a[1][:], A.add)
            nc.vector.tensor_tensor(u[:], u[:], a[2][:], A.add)
            nc.vector.tensor_tensor(u[:], u[:], a[3][:], A.add)
            with nc.allow_low_precision("fp16 pipeline, validated ~1e-3 vs fp64"):
                nc.vector.reciprocal(u[:], u[:])
            # Sum_c a_c = 1, so h = sum_c F[c,l]*a_c = sum_{c<3} (F[c,l]-F[3,l])*a_c
            # + F[3,l]: a3 is never needed, and the constant rides the ACT bias.
            for i in range(3):
                nc.vector.tensor_tensor(a[i][:], a[i][:], u[:], A.mult)

            x3 = xt[:].rearrange("p (l n) -> p l n", n=N)
            Y3 = Y[:].rearrange("p (s n) -> p s n", n=NP)

            # Stage 1: Y[:, s, l+n] += h_l[:, s, n] * x[:, l, n];  h_l -> DRAM
            Copy = mybir.ActivationFunctionType.Copy
            for l in range(L):
                # sum_c a_c = 1, so h = sum_{c<3} (F[c,l]-F[3,l])*a_c + F[3,l]:
                # one mul and one add fewer, constant rides the ACT seed bias.
                h = hp.tile([MSH, NS], f16, tag="h", name="ht")
                t1 = tp.tile([MSH, NS], f16, tag="tp", name="t1t")
                t2 = tp.tile([MSH, NS], f16, tag="tp", name="t2t")
                nc.scalar.activation(                                 # ACT seed + bias
                    h[:], a[0][:], Copy,
                    bias=float(F[3, l]), scale=float(F[0, l] - F[3, l]),
                )
                nc.scalar.mul(t1[:, :896], a[1][:, :896], float(F[1, l] - F[3, l]))
                nc.vector.tensor_scalar_mul(t1[:, 896:], a[1][:, 896:], float(F[1, l] - F[3, l]))
                nc.scalar.mul(t2[:], a[2][:], float(F[2, l] - F[3, l]))
                nc.vector.tensor_tensor(h[:], h[:], t1[:], A.add)
                nc.vector.tensor_tensor(h[:], h[:], t2[:], A.add)
                nc.sync.dma_start(hcache[l], h[:])
                p = pp.tile([MSH, NS], f16, tag="p", name="pt")
                xb = x3[:, l, :].unsqueeze(1).broadcast_to((MSH, S, N))
                nc.vector.tensor_tensor(
                    p[:].rearrange("p (s n) -> p s n", n=N),
                    h[:].rearrange("p (s n) -> p s n", n=N),
                    xb,
                    A.mult,
                )
                # Y-accumulate: alternate bands between the DMA engines (CCE
                # add on a strided dst) and the GpSimd/DVE stripe split, so no
                # single lane owns the dispersion add.
                if l % 3 != 2 and l < L - 1:
                    nc.gpsimd.dma_start(
                        Y3[:, :, l : l + N],
                        p[:].rearrange("p (s n) -> p s n", n=N),
                        accum_op=A.add,
                    )
                else:
                    g = 19 if l < L - 1 else 0
                    if g:
                        ysl = Y3[:, :g, l : l + N]
                        nc.gpsimd.tensor_tensor(
                            ysl, ysl,
                            p[:, : g * N].rearrange("p (s n) -> p s n", n=N),
                            A.add)
                    ysl2 = Y3[:, g:, l : l + N]
                    nc.vector.tensor_tensor(
                        ysl2, ysl2,
                        p[:, g * N :].rearrange("p (s n) -> p s n", n=N),
                        A.add)

            # Stage 2: X[:, l, n] = sum_s h_l[:, s, n] * Y[:, s, l+n]
            for l in range(L):
                h = main.tile([MSH, NL], f16, tag="x", bufs=2, name="hin")
                nc.sync.dma_start(h[:, :NS], hcache[l])
                t = pp.tile([MSH, NS], f16, tag="p", name="tt")
                nc.vector.tensor_tensor(
                    t[:].rearrange("p (s n) -> p s n", n=N),
                    h[:, :NS].rearrange("p (s n) -> p s n", n=N),
                    Y3[:, :, l : l + N],
                    A.mult,
                )
                # shot-sum tree over 22 contiguous stripes of N
                tv = t[:]
                nc.vector.tensor_tensor(
                    tv[:, : 5 * N], tv[:, : 5 * N], tv[:, 11 * N : 16 * N], A.add
                )
                nc.gpsimd.tensor_tensor(
                    tv[:, 5 * N : 10 * N], tv[:, 5 * N : 10 * N], tv[:, 16 * N : 21 * N], A.add
                )
                nc.vector.tensor_tensor(
                    tv[:, 10 * N : 11 * N], tv[:, 10 * N : 11 * N], tv[:, 21 * N : 22 * N], A.add
                )
                nc.gpsimd.tensor_tensor(
                    tv[:, : 5 * N], tv[:, : 5 * N], tv[:, 5 * N : 10 * N], A.add
                )
                nc.vector.tensor_tensor(
                    tv[:, : 2 * N], tv[:, : 2 * N], tv[:, 2 * N : 4 * N], A.add
                )
                nc.vector.tensor_tensor(tv[:, :N], tv[:, :N], tv[:, N : 2 * N], A.add)
                nc.vector.tensor_tensor(
                    tv[:, :N], tv[:, :N], tv[:, 4 * N : 5 * N], A.add
                )
                xol = tp.tile([MSH, N], f32, tag="xol", bufs=2, name="xolt")
                nc.vector.tensor_tensor(
                    xol[:], tv[:, :N], tv[:, 10 * N : 11 * N], A.add
                )
                nc.sync.dma_start(out[:, l * N : (l + 1) * N], xol[:])

    nc.compile()
    return nc


def _get_nc():
    global _NC
    if _NC is None:
        _NC = _build()
    return _NC


def _make_in_maps(x, wr, wg, wb, wc):
    x = np.asarray(x, dtype=np.float32)
    ws = [np.asarray(w, dtype=np.float32).reshape(M, M, S) for w in (wr, wg, wb, wc)]
    in_maps = []
    for core in range(NCORES):
        b, mh = divmod(core, 2)
        rows = slice(mh * MSH, (mh + 1) * MSH)
        xs = x[b, rows].transpose(0, 2, 1)            # (MSH, L, N)
        m = {"x16": np.ascontiguousarray(xs).reshape(MSH, NL).astype(np.float16)}
        for i, w in enumerate(ws):
            wsb = w[rows].transpose(0, 2, 1)          # (MSH, S, N)
            m[f"w{i}"] = np.ascontiguousarray(wsb).reshape(MSH, NS).astype(np.float16)
        in_maps.append(m)
    return in_maps


def _run_shards(in_maps):
    from concourse.bass_utils import run_bass_kernel_spmd

    nc = _get_nc()
    return run_bass_kernel_spmd(nc, in_maps, list(range(NCORES)))


def kernel(x, wr, wg, wb, wc):
    res = _run_shards(_make_in_maps(x, wr, wg, wb, wc))
    X = np.empty((B, M, N, L), dtype=np.float32)
    for core in range(NCORES):
        b, mh = divmod(core, 2)
        xo = res.results[core]["out"].reshape(MSH, L, N).transpose(0, 2, 1)
        X[b, mh * MSH : (mh + 1) * MSH] = xo
    return X / X.max()


def estimate_ns() -> float:
    """Single-core cost-model estimate of the kernel duration (ns)."""
    from concourse.timeline_sim import TimelineSim

    return TimelineSim(_get_nc()).simulate()



# revision 7
# speedup vs baseline: 1.0050x; 1.0050x over previous
"""CASSI colored-aperture layer (nn_CASSI_layer_Colored) on 8 Trainium2 NeuronCores.

Reference semantics (B=4, M=N=KERN=256, L=24 bands, S=22 shots):
    H[m,n,l,s] = (wr*fr[l] + wg*fg[l] + wb*fb[l] + wc*fc[l]) / (wr+wg+wb+wc)
    Y[b,m,n',s] = sum_l H[m,n'-l,l,s] * x[b,m,n'-l,l]          (dispersion shift-sum)
    X[b,m,n,l]  = sum_s H[m,n,l,s] * Y[b,m,n+l,s]              (adjoint + shot sum)
    out = X / max(X)

Sharding: data-parallel over (batch b, row-half mh): 4 x 2 = 8 cores.  Rows m
never couple, so each core computes 128 rows of one batch independently; only
the final global max couples shards (host side, after the gather).

Per-core mapping: partitions = 128 m-rows; free dims are s-major (s, n) so the
dispersion shift n -> n+l is a free-dim offset, the broadcast of x over s is a
stride-0 outer AP dim (dense innermost keeps DVE 2x mode), and the shot-sum
becomes contiguous stripe-halving adds.  Pipeline is fp16 (~1e-3 max rel err
vs fp64, validated).  Per band l:
  stage 1: h_l = sum_c F[c,l]*a_c (ScalarE seeds + partials, DVE/GpSimd adds),
           Y[:, l:l+N] += h_l * x[:, l-bcast]: on 2/3 of bands the add runs as
           a DMA-engine CCE accumulate (dst AP must stay strided: contiguous
           dst APs silently drop the accumulate), relieving GpSimd/DVE; the
           rest keep the GpSimd/DVE stripe split.  h_l spilled to DRAM.
  stage 2: h_l reloaded (DMA, hidden), t = h_l * Y[:, l:l+N] (DVE),
           X[:, l] = stripe-tree shot sum (GpSimd first level, DVE rest)
"""

import numpy as np

B, M, N, L, S = 4, 256, 256, 24, 22
MSH = M // 2                     # rows per core
NCORES = 8
NS, NL = N * S, N * L
NP = N + L - 1                   # 279 shifted columns
YW = NP * S                      # Y free width (s-major: s outer, n' inner)


def _bases() -> np.ndarray:
    """(4, L) color responses paired row-wise with (wr, wg, wb, wc)."""
    wl = np.linspace(400.0, 700.0, L)

    def g(mu: float, sig: float) -> np.ndarray:
        return np.exp(-0.5 * ((wl - mu) / sig) ** 2)

    # reference: H = wr*f620 + wg*f550 + wb*f450 + wc*f500 (fr,fg,fc,fb = 620,550,500,450)
    return np.stack([g(620.0, 50.0), g(550.0, 50.0), g(450.0, 50.0), g(500.0, 50.0)])


_NC = None


def _build():
    import concourse.bacc as bacc
    import concourse.mybir as mybir
    import concourse.tile as tile

    f16, f32 = mybir.dt.float16, mybir.dt.float32
    A = mybir.AluOpType
    F = _bases()

    nc = bacc.Bacc("TRN2", target_bir_lowering=False, debug=False, num_devices=NCORES)
    xin = nc.declare_dram_parameter("x16", [MSH, NL], f16, isOutput=False)   # (l, n)
    wins = [
        nc.declare_dram_parameter(f"w{i}", [MSH, NS], f16, isOutput=False)   # (s, n)
        for i in range(4)
    ]
    out = nc.declare_dram_parameter("out", [MSH, NL], f32, isOutput=True)    # (l, n)
    hcache = nc.dram_tensor("hcache", [L, MSH, NS], f16)

    with tile.TileContext(nc) as tc:
        with (
            tc.tile_pool(name="main", bufs=1) as main,
            tc.tile_pool(name="hp", bufs=3) as hp,
            tc.tile_pool(name="tp", bufs=4) as tp,
            tc.tile_pool(name="pp", bufs=2) as pp,
        ):
            a = [main.tile([MSH, NS], f16, tag=f"a{i}", name=f"a{i}") for i in range(4)]
            xt = main.tile([MSH, NL], f16, tag="x", bufs=2, name="xt")
            Y = main.tile([MSH, YW], f16, tag="Y", name="Yt")

            for i in range(4):
                nc.sync.dma_start(a[i][:], wins[i][:])
            nc.sync.dma_start(xt[:], xin[:])
            nc.gpsimd.memset(Y[:], 0.0)

            # a_c = w_c / (wr+wg+wb+wc)
            u = hp.tile([MSH, NS], f16, tag="h", name="ut")
            nc.vector.tensor_tensor(u[:], a[0][:], a[1][:], A.add)
            nc.vector.tensor_tensor(u[:], u[:], a[2][:], A.add)
            nc.vector.tensor_tensor(u[:], u[:], a[3][:], A.add)
            with nc.allow_low_precision("fp16 pipeline, validated ~1e-3 vs fp64"):
                nc.vector.reciprocal(u[:], u[:])
            # Sum_c a_c = 1, so h = sum_c F[c,l]*a_c = sum_{c<3} (F[c,l]-F[3,l])*a_c
            # + F[3,l]: a3 is never needed, and the constant rides the ACT bias.
            for i in range(3):
                nc.vector.tensor_tensor(a[i][:], a[i][:], u[:], A.mult)

            x3 = xt[:].rearrange("p (l n) -> p l n", n=N)
            Y3 = Y[:].rearrange("p (s n) -> p s n", n=NP)

            # Stage 1: Y[:, s, l+n] += h_l[:, s, n] * x[:, l, n];  h_l -> DRAM
            Copy = mybir.ActivationFunctionType.Copy
            for l in range(L):
                # sum_c a_c = 1, so h = sum_{c<3} (F[c,l]-F[3,l])*a_c + F[3,l]:
                # one mul and one add fewer, constant rides the ACT seed bias.
                h = hp.tile([MSH, NS], f16, tag="h", name="ht")
                t1 = tp.tile([MSH, NS], f16, tag="tp", name="t1t")
                t2 = tp.tile([MSH, NS], f16, tag="tp", name="t2t")
                nc.scalar.activation(                                 # ACT seed + bias
                    h[:], a[0][:], Copy,
                    bias=float(F[3, l]), scale=float(F[0, l] - F[3, l]),
                )
                nc.scalar.mul(t1[:, :896], a[1][:, :896], float(F[1, l] - F[3, l]))
                nc.vector.tensor_scalar_mul(t1[:, 896:], a[1][:, 896:], float(F[1, l] - F[3, l]))
                nc.scalar.mul(t2[:, : 21 * N], a[2][:, : 21 * N], float(F[2, l] - F[3, l]))
                nc.vector.tensor_scalar_mul(t2[:, 21 * N :], a[2][:, 21 * N :], float(F[2, l] - F[3, l]))
                nc.vector.tensor_tensor(h[:], h[:], t1[:], A.add)
                nc.vector.tensor_tensor(h[:], h[:], t2[:], A.add)
                nc.sync.dma_start(hcache[l], h[:])
                p = pp.tile([MSH, NS], f16, tag="p", name="pt")
                xb = x3[:, l, :].unsqueeze(1).broadcast_to((MSH, S, N))
                nc.vector.tensor_tensor(
                    p[:].rearrange("p (s n) -> p s n", n=N),
                    h[:].rearrange("p (s n) -> p s n", n=N),
                    xb,
                    A.mult,
                )
                # Y-accumulate: alternate bands between the DMA engines (CCE
                # add on a strided dst) and the GpSimd/DVE stripe split, so no
                # single lane owns the dispersion add.
                if l % 3 != 2 and l < L - 1:
                    nc.gpsimd.dma_start(
                        Y3[:, :, l : l + N],
                        p[:].rearrange("p (s n) -> p s n", n=N),
                        accum_op=A.add,
                    )
                else:
                    g = 19 if l < L - 1 else 0
                    if g:
                        ysl = Y3[:, :g, l : l + N]
                        nc.gpsimd.tensor_tensor(
                            ysl, ysl,
                            p[:, : g * N].rearrange("p (s n) -> p s n", n=N),
                            A.add)
                    ysl2 = Y3[:, g:, l : l + N]
                    nc.vector.tensor_tensor(
                        ysl2, ysl2,
                        p[:, g * N :].rearrange("p (s n) -> p s n", n=N),
                        A.add)

            # Stage 2: X[:, l, n] = sum_s h_l[:, s, n] * Y[:, s, l+n]
            for l in range(L):
                h = main.tile([MSH, NL], f16, tag="x", bufs=2, name="hin")
                nc.sync.dma_start(h[:, :NS], hcache[l])
                t = pp.tile([MSH, NS], f16, tag="p", name="tt")
                nc.vector.tensor_tensor(
                    t[:].rearrange("p (s n) -> p s n", n=N),
                    h[:, :NS].rearrange("p (s n) -> p s n", n=N),
                    Y3[:, :, l : l + N],
                    A.mult,
                )
                # shot-sum tree over 22 contiguous stripes of N
                tv = t[:]
                nc.vector.tensor_tensor(
                    tv[:, : 5 * N], tv[:, : 5 * N], tv[:, 11 * N : 16 * N], A.add
                )
                nc.gpsimd.tensor_tensor(
                    tv[:, 5 * N : 10 * N], tv[:, 5 * N : 10 * N], tv[:, 16 * N : 21 * N], A.add
                )
                nc.vector.tensor_tensor(
                    tv[:, 10 * N : 11 * N], tv[:, 10 * N : 11 * N], tv[:, 21 * N : 22 * N], A.add
                )
                nc.gpsimd.tensor_tensor(
                    tv[:, : 5 * N], tv[:, : 5 * N], tv[:, 5 * N : 10 * N], A.add
                )
                nc.vector.tensor_tensor(
                    tv[:, : 2 * N], tv[:, : 2 * N], tv[:, 2 * N : 4 * N], A.add
                )
                nc.vector.tensor_tensor(tv[:, :N], tv[:, :N], tv[:, N : 2 * N], A.add)
                nc.vector.tensor_tensor(
                    tv[:, :N], tv[:, :N], tv[:, 4 * N : 5 * N], A.add
                )
                xol = tp.tile([MSH, N], f32, tag="xol", bufs=2, name="xolt")
                nc.vector.tensor_tensor(
                    xol[:], tv[:, :N], tv[:, 10 * N : 11 * N], A.add
                )
                nc.sync.dma_start(out[:, l * N : (l + 1) * N], xol[:])

    nc.compile()
    return nc


def _get_nc():
    global _NC
    if _NC is None:
        _NC = _build()
    return _NC


def _make_in_maps(x, wr, wg, wb, wc):
    x = np.asarray(x, dtype=np.float32)
    ws = [np.asarray(w, dtype=np.float32).reshape(M, M, S) for w in (wr, wg, wb, wc)]
    in_maps = []
    for core in range(NCORES):
        b, mh = divmod(core, 2)
        rows = slice(mh * MSH, (mh + 1) * MSH)
        xs = x[b, rows].transpose(0, 2, 1)            # (MSH, L, N)
        m = {"x16": np.ascontiguousarray(xs).reshape(MSH, NL).astype(np.float16)}
        for i, w in enumerate(ws):
            wsb = w[rows].transpose(0, 2, 1)          # (MSH, S, N)
            m[f"w{i}"] = np.ascontiguousarray(wsb).reshape(MSH, NS).astype(np.float16)
        in_maps.append(m)
    return in_maps


def _run_shards(in_maps):
    from concourse.bass_utils import run_bass_kernel_spmd

    nc = _get_nc()
    return run_bass_kernel_spmd(nc, in_maps, list(range(NCORES)))


def kernel(x, wr, wg, wb, wc):
    res = _run_shards(_make_in_maps(x, wr, wg, wb, wc))
    X = np.empty((B, M, N, L), dtype=np.float32)
    for core in range(NCORES):
        b, mh = divmod(core, 2)
        xo = res.results[core]["out"].reshape(MSH, L, N).transpose(0, 2, 1)
        X[b, mh * MSH : (mh + 1) * MSH] = xo
    return X / X.max()


def estimate_ns() -> float:
    """Single-core cost-model estimate of the kernel duration (ns)."""
    from concourse.timeline_sim import TimelineSim

    return TimelineSim(_get_nc()).simulate()



# revision 13
# speedup vs baseline: 1.0554x; 1.0501x over previous
"""CASSI colored-aperture layer (nn_CASSI_layer_Colored) on 8 Trainium2 NeuronCores.

Reference semantics (B=4, M=N=KERN=256, L=24 bands, S=22 shots):
    H[m,n,l,s] = (wr*fr[l] + wg*fg[l] + wb*fb[l] + wc*fc[l]) / (wr+wg+wb+wc)
    Y[b,m,n',s] = sum_l H[m,n'-l,l,s] * x[b,m,n'-l,l]          (dispersion shift-sum)
    X[b,m,n,l]  = sum_s H[m,n,l,s] * Y[b,m,n+l,s]              (adjoint + shot sum)
    out = X / max(X)

Sharding: data-parallel over (batch b, row-half mh): 4 x 2 = 8 cores.  Rows m
never couple, so each core computes 128 rows of one batch independently; only
the final global max couples shards (host side, after the gather).

Per-core mapping: partitions = 128 m-rows; free dims are s-major (s, n) so the
dispersion shift n -> n+l is a free-dim offset, the broadcast of x over s is a
stride-0 outer AP dim (dense innermost keeps DVE 2x mode), and the shot-sum
becomes contiguous stripe-halving adds.  Pipeline is fp16 (~1e-3 max rel err
vs fp64, validated).  Per band l:
  stage 1: h_l = sum_c F[c,l]*a_c (ScalarE seeds + partials, DVE/GpSimd adds),
           Y[:, l:l+N] += h_l * x[:, l-bcast]: on 2/3 of bands the add runs as
           a DMA-engine CCE accumulate (dst AP must stay strided: contiguous
           dst APs silently drop the accumulate), relieving GpSimd/DVE; the
           rest keep the GpSimd/DVE stripe split.  h_l spilled to DRAM.
  stage 2: h_l reloaded (DMA, hidden), t = h_l * Y[:, l:l+N] (DVE),
           X[:, l] = stripe-tree shot sum (GpSimd first level, DVE rest)
"""

import numpy as np

B, M, N, L, S = 4, 256, 256, 24, 22
MSH = M // 2                     # rows per core
NCORES = 8
NS, NL = N * S, N * L
NP = N + L - 1                   # 279 shifted columns
YW = NP * S                      # Y free width (s-major: s outer, n' inner)


def _bases() -> np.ndarray:
    """(4, L) color responses paired row-wise with (wr, wg, wb, wc)."""
    wl = np.linspace(400.0, 700.0, L)

    def g(mu: float, sig: float) -> np.ndarray:
        return np.exp(-0.5 * ((wl - mu) / sig) ** 2)

    # reference: H = wr*f620 + wg*f550 + wb*f450 + wc*f500 (fr,fg,fc,fb = 620,550,500,450)
    return np.stack([g(620.0, 50.0), g(550.0, 50.0), g(450.0, 50.0), g(500.0, 50.0)])


_NC = None


def _build():
    import concourse.bacc as bacc
    import concourse.mybir as mybir
    import concourse.tile as tile

    f16, f32 = mybir.dt.float16, mybir.dt.float32
    A = mybir.AluOpType
    F = _bases()

    nc = bacc.Bacc("TRN2", target_bir_lowering=False, debug=False, num_devices=NCORES)
    xin = nc.declare_dram_parameter("x16", [MSH, NL], f16, isOutput=False)   # (l, n)
    wins = [
        nc.declare_dram_parameter(f"w{i}", [MSH, NS], f16, isOutput=False)   # (s, n)
        for i in range(4)
    ]
    out = nc.declare_dram_parameter("out", [MSH, NL], f16, isOutput=True)    # (l, n)
    hcache = nc.dram_tensor("hcache", [L, MSH, NS], f16)

    with tile.TileContext(nc) as tc:
        with (
            tc.tile_pool(name="main", bufs=1) as main,
            tc.tile_pool(name="hp", bufs=3) as hp,
            tc.tile_pool(name="tp", bufs=4) as tp,
            tc.tile_pool(name="pp", bufs=2) as pp,
        ):
            a = [main.tile([MSH, NS], f16, tag=f"a{i}", name=f"a{i}") for i in range(4)]
            xt = main.tile([MSH, NL], f16, tag="x", bufs=2, name="xt")
            Y = main.tile([MSH, YW], f16, tag="Y", name="Yt")

            for i in range(4):
                nc.sync.dma_start(a[i][:], wins[i][:])
            nc.sync.dma_start(xt[:], xin[:])
            nc.gpsimd.memset(Y[:], 0.0)

            # a_c = w_c / (wr+wg+wb+wc)
            u = hp.tile([MSH, NS], f16, tag="h", name="ut")
            nc.vector.tensor_tensor(u[:], a[0][:], a[1][:], A.add)
            nc.vector.tensor_tensor(u[:], u[:], a[2][:], A.add)
            nc.vector.tensor_tensor(u[:], u[:], a[3][:], A.add)
            with nc.allow_low_precision("fp16 pipeline, validated ~1e-3 vs fp64"):
                nc.vector.reciprocal(u[:], u[:])
            # Sum_c a_c = 1, so h = sum_c F[c,l]*a_c = sum_{c<3} (F[c,l]-F[3,l])*a_c
            # + F[3,l]: a3 is never needed, and the constant rides the ACT bias.
            for i in range(3):
                nc.vector.tensor_tensor(a[i][:], a[i][:], u[:], A.mult)

            x3 = xt[:].rearrange("p (l n) -> p l n", n=N)
            Y3 = Y[:].rearrange("p (s n) -> p s n", n=NP)

            # Stage 1: Y[:, s, l+n] += h_l[:, s, n] * x[:, l, n];  h_l -> DRAM
            Copy = mybir.ActivationFunctionType.Copy
            for l in range(L):
                # sum_c a_c = 1, so h = sum_{c<3} (F[c,l]-F[3,l])*a_c + F[3,l]:
                # one mul and one add fewer, constant rides the ACT seed bias.
                h = hp.tile([MSH, NS], f16, tag="h", name="ht")
                t1 = tp.tile([MSH, NS], f16, tag="tp", name="t1t")
                t2 = tp.tile([MSH, NS], f16, tag="tp", name="t2t")
                nc.scalar.activation(                                 # ACT seed + bias
                    h[:], a[0][:], Copy,
                    bias=float(F[3, l]), scale=float(F[0, l] - F[3, l]),
                )
                nc.scalar.mul(t1[:, :896], a[1][:, :896], float(F[1, l] - F[3, l]))
                nc.vector.tensor_scalar_mul(t1[:, 896:], a[1][:, 896:], float(F[1, l] - F[3, l]))
                nc.scalar.mul(t2[:, : 16 * N], a[2][:, : 16 * N], float(F[2, l] - F[3, l]))
                nc.vector.tensor_scalar_mul(t2[:, 16 * N :], a[2][:, 16 * N :], float(F[2, l] - F[3, l]))
                nc.vector.tensor_tensor(h[:, : 15 * N], h[:, : 15 * N], t1[:, : 15 * N], A.add)
                nc.gpsimd.tensor_tensor(h[:, 15 * N :], h[:, 15 * N :], t1[:, 15 * N :], A.add)
                nc.vector.tensor_tensor(h[:], h[:], t2[:], A.add)
                nc.sync.dma_start(hcache[l], h[:])
                p = pp.tile([MSH, NS], f16, tag="p", name="pt")
                xb = x3[:, l, :].unsqueeze(1).broadcast_to((MSH, S, N))
                nc.vector.tensor_tensor(
                    p[:].rearrange("p (s n) -> p s n", n=N),
                    h[:].rearrange("p (s n) -> p s n", n=N),
                    xb,
                    A.mult,
                )
                # Y-accumulate: alternate bands between the DMA engines (CCE
                # add on a strided dst) and the GpSimd/DVE stripe split, so no
                # single lane owns the dispersion add.
                if l % 3 != 2 and l < L - 1:
                    nc.gpsimd.dma_start(
                        Y3[:, :, l : l + N],
                        p[:].rearrange("p (s n) -> p s n", n=N),
                        accum_op=A.add,
                    )
                else:
                    g = 19 if l < L - 1 else 0
                    if g:
                        ysl = Y3[:, :g, l : l + N]
                        nc.gpsimd.tensor_tensor(
                            ysl, ysl,
                            p[:, : g * N].rearrange("p (s n) -> p s n", n=N),
                            A.add)
                    ysl2 = Y3[:, g:, l : l + N]
                    nc.vector.tensor_tensor(
                        ysl2, ysl2,
                        p[:, g * N :].rearrange("p (s n) -> p s n", n=N),
                        A.add)

            # Stage 2: X[:, l, n] = sum_s h_l[:, s, n] * Y[:, s, l+n]
            for l in range(L):
                h = main.tile([MSH, NL], f16, tag="x", bufs=2, name="hin")
                nc.sync.dma_start(h[:, :NS], hcache[l])
                t = pp.tile([MSH, NS], f16, tag="p", name="tt")
                nc.vector.tensor_tensor(
                    t[:].rearrange("p (s n) -> p s n", n=N),
                    h[:, :NS].rearrange("p (s n) -> p s n", n=N),
                    Y3[:, :, l : l + N],
                    A.mult,
                )
                # shot-sum tree over 22 contiguous stripes of N
                tv = t[:]
                nc.vector.tensor_tensor(
                    tv[:, : 5 * N], tv[:, : 5 * N], tv[:, 11 * N : 16 * N], A.add
                )
                nc.gpsimd.tensor_tensor(
                    tv[:, 5 * N : 9 * N], tv[:, 5 * N : 9 * N], tv[:, 16 * N : 20 * N], A.add
                )
                nc.vector.tensor_tensor(
                    tv[:, 9 * N : 10 * N], tv[:, 9 * N : 10 * N], tv[:, 20 * N : 21 * N], A.add
                )
                nc.vector.tensor_tensor(
                    tv[:, 10 * N : 11 * N], tv[:, 10 * N : 11 * N], tv[:, 21 * N : 22 * N], A.add
                )
                nc.gpsimd.tensor_tensor(
                    tv[:, : 5 * N], tv[:, : 5 * N], tv[:, 5 * N : 10 * N], A.add
                )
                nc.vector.tensor_tensor(
                    tv[:, : 2 * N], tv[:, : 2 * N], tv[:, 2 * N : 4 * N], A.add
                )
                nc.vector.tensor_tensor(tv[:, :N], tv[:, :N], tv[:, N : 2 * N], A.add)
                nc.vector.tensor_tensor(
                    tv[:, :N], tv[:, :N], tv[:, 4 * N : 5 * N], A.add
                )
                xol = tp.tile([MSH, N], f16, tag="xol", bufs=2, name="xolt")
                nc.vector.tensor_tensor(
                    xol[:], tv[:, :N], tv[:, 10 * N : 11 * N], A.add
                )
                nc.sync.dma_start(out[:, l * N : (l + 1) * N], xol[:])

    nc.compile()
    return nc


def _get_nc():
    global _NC
    if _NC is None:
        _NC = _build()
    return _NC


def _make_in_maps(x, wr, wg, wb, wc):
    x = np.asarray(x, dtype=np.float32)
    ws = [np.asarray(w, dtype=np.float32).reshape(M, M, S) for w in (wr, wg, wb, wc)]
    in_maps = []
    for core in range(NCORES):
        b, mh = divmod(core, 2)
        rows = slice(mh * MSH, (mh + 1) * MSH)
        xs = x[b, rows].transpose(0, 2, 1)            # (MSH, L, N)
        m = {"x16": np.ascontiguousarray(xs).reshape(MSH, NL).astype(np.float16)}
        for i, w in enumerate(ws):
            wsb = w[rows].transpose(0, 2, 1)          # (MSH, S, N)
            m[f"w{i}"] = np.ascontiguousarray(wsb).reshape(MSH, NS).astype(np.float16)
        in_maps.append(m)
    return in_maps


def _run_shards(in_maps):
    from concourse.bass_utils import run_bass_kernel_spmd

    nc = _get_nc()
    return run_bass_kernel_spmd(nc, in_maps, list(range(NCORES)))


def kernel(x, wr, wg, wb, wc):
    res = _run_shards(_make_in_maps(x, wr, wg, wb, wc))
    X = np.empty((B, M, N, L), dtype=np.float32)
    for core in range(NCORES):
        b, mh = divmod(core, 2)
        xo = res.results[core]["out"].astype(np.float32).reshape(MSH, L, N).transpose(0, 2, 1)
        X[b, mh * MSH : (mh + 1) * MSH] = xo
    return X / X.max()


def estimate_ns() -> float:
    """Single-core cost-model estimate of the kernel duration (ns)."""
    from concourse.timeline_sim import TimelineSim

    return TimelineSim(_get_nc()).simulate()



# revision 18
# speedup vs baseline: 1.0937x; 1.0363x over previous
"""CASSI colored-aperture layer (nn_CASSI_layer_Colored) on 8 Trainium2 NeuronCores.

Reference semantics (B=4, M=N=KERN=256, L=24 bands, S=22 shots):
    H[m,n,l,s] = (wr*fr[l] + wg*fg[l] + wb*fb[l] + wc*fc[l]) / (wr+wg+wb+wc)
    Y[b,m,n',s] = sum_l H[m,n'-l,l,s] * x[b,m,n'-l,l]          (dispersion shift-sum)
    X[b,m,n,l]  = sum_s H[m,n,l,s] * Y[b,m,n+l,s]              (adjoint + shot sum)
    out = X / max(X)

Sharding: data-parallel over (batch b, row-half mh): 4 x 2 = 8 cores.  Rows m
never couple, so each core computes 128 rows of one batch independently; only
the final global max couples shards (host side, after the gather).

Per-core mapping: partitions = 128 m-rows; free dims are s-major (s, n) so the
dispersion shift n -> n+l is a free-dim offset, the broadcast of x over s is a
stride-0 outer AP dim (dense innermost keeps DVE 2x mode), and the shot-sum
becomes contiguous stripe-halving adds.  Pipeline is fp16 (~1e-3 max rel err
vs fp64, validated).  Per band l:
  stage 1: h_l = sum_c F[c,l]*a_c (ScalarE seeds + partials, DVE/GpSimd adds),
           Y[:, l:l+N] += h_l * x[:, l-bcast]: on 2/3 of bands the add runs as
           a DMA-engine CCE accumulate (dst AP must stay strided: contiguous
           dst APs silently drop the accumulate), relieving GpSimd/DVE; the
           rest keep the GpSimd/DVE stripe split.  h_l spilled to DRAM.
  stage 2: h_l reloaded (DMA, hidden), t = h_l * Y[:, l:l+N] (DVE),
           X[:, l] = stripe-tree shot sum (GpSimd first level, DVE rest)
"""

import numpy as np

B, M, N, L, S = 4, 256, 256, 24, 22
MSH = M // 2                     # rows per core
NCORES = 8
NS, NL = N * S, N * L
NP = N + L - 1                   # 279 shifted columns
YW = NP * S                      # Y free width (s-major: s outer, n' inner)


def _bases() -> np.ndarray:
    """(4, L) color responses paired row-wise with (wr, wg, wb, wc)."""
    wl = np.linspace(400.0, 700.0, L)

    def g(mu: float, sig: float) -> np.ndarray:
        return np.exp(-0.5 * ((wl - mu) / sig) ** 2)

    # reference: H = wr*f620 + wg*f550 + wb*f450 + wc*f500 (fr,fg,fc,fb = 620,550,500,450)
    return np.stack([g(620.0, 50.0), g(550.0, 50.0), g(450.0, 50.0), g(500.0, 50.0)])


_NC = None


def _build():
    import concourse.bacc as bacc
    import concourse.mybir as mybir
    import concourse.tile as tile

    f16, f32 = mybir.dt.float16, mybir.dt.float32
    A = mybir.AluOpType
    F = _bases()

    nc = bacc.Bacc("TRN2", target_bir_lowering=False, debug=False, num_devices=NCORES)
    xin = nc.declare_dram_parameter("x16", [MSH, NL], f16, isOutput=False)   # (l, n)
    wins = [
        nc.declare_dram_parameter(f"w{i}", [MSH, NS], f16, isOutput=False)   # (s, n)
        for i in range(4)
    ]
    out = nc.declare_dram_parameter("out", [MSH, NL], f16, isOutput=True)    # (l, n)
    hcache = nc.dram_tensor("hcache", [L, MSH, NS], f16)

    with tile.TileContext(nc) as tc:
        with (
            tc.tile_pool(name="main", bufs=1) as main,
            tc.tile_pool(name="hp", bufs=3) as hp,
            tc.tile_pool(name="tp", bufs=4) as tp,
            tc.tile_pool(name="pp", bufs=2) as pp,
        ):
            a = [main.tile([MSH, NS], f16, tag=f"a{i}", name=f"a{i}") for i in range(4)]
            xt = main.tile([MSH, NL], f16, tag="x", bufs=2, name="xt")
            Y = main.tile([MSH, YW], f16, tag="Y", name="Yt")

            for i in range(4):
                nc.sync.dma_start(a[i][:], wins[i][:])
            nc.sync.dma_start(xt[:], xin[:])
            nc.gpsimd.memset(Y[:], 0.0)

            # a_c = w_c / (wr+wg+wb+wc)
            u = hp.tile([MSH, NS], f16, tag="h", name="ut")
            nc.vector.tensor_tensor(u[:], a[0][:], a[1][:], A.add)
            nc.vector.tensor_tensor(u[:], u[:], a[2][:], A.add)
            nc.vector.tensor_tensor(u[:], u[:], a[3][:], A.add)
            with nc.allow_low_precision("fp16 pipeline, validated ~1e-3 vs fp64"):
                nc.vector.reciprocal(u[:], u[:])
            # Sum_c a_c = 1, so h = sum_c F[c,l]*a_c = sum_{c<3} (F[c,l]-F[3,l])*a_c
            # + F[3,l]: a3 is never needed, and the constant rides the ACT bias.
            for i in range(3):
                nc.vector.tensor_tensor(a[i][:], a[i][:], u[:], A.mult)

            x3 = xt[:].rearrange("p (l n) -> p l n", n=N)
            Y3 = Y[:].rearrange("p (s n) -> p s n", n=NP)

            # Stage 1: Y[:, s, l+n] += h_l[:, s, n] * x[:, l, n];  h_l -> DRAM
            Copy = mybir.ActivationFunctionType.Copy
            for l in range(L):
                # sum_c a_c = 1, so h = sum_{c<3} (F[c,l]-F[3,l])*a_c + F[3,l]:
                # one mul and one add fewer, constant rides the ACT seed bias.
                h = hp.tile([MSH, NS], f16, tag="h", name="ht")
                t1 = tp.tile([MSH, NS], f16, tag="tp", name="t1t")
                t2 = tp.tile([MSH, NS], f16, tag="tp", name="t2t")
                nc.scalar.activation(                                 # ACT seed + bias
                    h[:], a[0][:], Copy,
                    bias=float(F[3, l]), scale=float(F[0, l] - F[3, l]),
                )
                nc.scalar.mul(t1[:, :896], a[1][:, :896], float(F[1, l] - F[3, l]))
                nc.vector.tensor_scalar_mul(t1[:, 896:], a[1][:, 896:], float(F[1, l] - F[3, l]))
                nc.scalar.mul(t2[:, : 15 * N], a[2][:, : 15 * N], float(F[2, l] - F[3, l]))
                nc.vector.tensor_scalar_mul(t2[:, 15 * N :], a[2][:, 15 * N :], float(F[2, l] - F[3, l]))
                nc.vector.tensor_tensor(h[:, : 10 * N], h[:, : 10 * N], t1[:, : 10 * N], A.add)
                nc.gpsimd.tensor_tensor(h[:, 10 * N :], h[:, 10 * N :], t1[:, 10 * N :], A.add)
                nc.vector.tensor_tensor(h[:], h[:], t2[:], A.add)
                nc.sync.dma_start(hcache[l], h[:])
                p = pp.tile([MSH, NS], f16, tag="p", name="pt")
                xb = x3[:, l, :].unsqueeze(1).broadcast_to((MSH, S, N))
                nc.vector.tensor_tensor(
                    p[:].rearrange("p (s n) -> p s n", n=N),
                    h[:].rearrange("p (s n) -> p s n", n=N),
                    xb,
                    A.mult,
                )
                # Y-accumulate: alternate bands between the DMA engines (CCE
                # add on a strided dst) and the GpSimd/DVE stripe split, so no
                # single lane owns the dispersion add.
                if l < L - 1:
                    nc.gpsimd.dma_start(
                        Y3[:, :, l : l + N],
                        p[:].rearrange("p (s n) -> p s n", n=N),
                        accum_op=A.add,
                    )
                else:
                    g = 19 if l < L - 1 else 0
                    if g:
                        ysl = Y3[:, :g, l : l + N]
                        nc.gpsimd.tensor_tensor(
                            ysl, ysl,
                            p[:, : g * N].rearrange("p (s n) -> p s n", n=N),
                            A.add)
                    ysl2 = Y3[:, g:, l : l + N]
                    nc.vector.tensor_tensor(
                        ysl2, ysl2,
                        p[:, g * N :].rearrange("p (s n) -> p s n", n=N),
                        A.add)

            # Stage 2: X[:, l, n] = sum_s h_l[:, s, n] * Y[:, s, l+n]
            for l in range(L):
                h = main.tile([MSH, NL], f16, tag="x", bufs=2, name="hin")
                nc.sync.dma_start(h[:, :NS], hcache[l])
                t = pp.tile([MSH, NS], f16, tag="p", name="tt")
                nc.vector.tensor_tensor(
                    t[:].rearrange("p (s n) -> p s n", n=N),
                    h[:, :NS].rearrange("p (s n) -> p s n", n=N),
                    Y3[:, :, l : l + N],
                    A.mult,
                )
                # shot-sum tree over 22 contiguous stripes of N
                tv = t[:]
                nc.vector.tensor_tensor(
                    tv[:, : 5 * N], tv[:, : 5 * N], tv[:, 11 * N : 16 * N], A.add
                )
                nc.gpsimd.tensor_tensor(
                    tv[:, 5 * N : 9 * N], tv[:, 5 * N : 9 * N], tv[:, 16 * N : 20 * N], A.add
                )
                nc.vector.tensor_tensor(
                    tv[:, 9 * N : 10 * N], tv[:, 9 * N : 10 * N], tv[:, 20 * N : 21 * N], A.add
                )
                nc.vector.tensor_tensor(
                    tv[:, 10 * N : 11 * N], tv[:, 10 * N : 11 * N], tv[:, 21 * N : 22 * N], A.add
                )
                nc.gpsimd.tensor_tensor(
                    tv[:, : 5 * N], tv[:, : 5 * N], tv[:, 5 * N : 10 * N], A.add
                )
                nc.vector.tensor_tensor(
                    tv[:, : 2 * N], tv[:, : 2 * N], tv[:, 2 * N : 4 * N], A.add
                )
                nc.vector.tensor_tensor(tv[:, :N], tv[:, :N], tv[:, N : 2 * N], A.add)
                nc.vector.tensor_tensor(
                    tv[:, :N], tv[:, :N], tv[:, 4 * N : 5 * N], A.add
                )
                xol = tp.tile([MSH, N], f16, tag="xol", bufs=2, name="xolt")
                nc.vector.tensor_tensor(
                    xol[:], tv[:, :N], tv[:, 10 * N : 11 * N], A.add
                )
                nc.sync.dma_start(out[:, l * N : (l + 1) * N], xol[:])

    nc.compile()
    return nc


def _get_nc():
    global _NC
    if _NC is None:
        _NC = _build()
    return _NC


def _make_in_maps(x, wr, wg, wb, wc):
    x = np.asarray(x, dtype=np.float32)
    ws = [np.asarray(w, dtype=np.float32).reshape(M, M, S) for w in (wr, wg, wb, wc)]
    in_maps = []
    for core in range(NCORES):
        b, mh = divmod(core, 2)
        rows = slice(mh * MSH, (mh + 1) * MSH)
        xs = x[b, rows].transpose(0, 2, 1)            # (MSH, L, N)
        m = {"x16": np.ascontiguousarray(xs).reshape(MSH, NL).astype(np.float16)}
        for i, w in enumerate(ws):
            wsb = w[rows].transpose(0, 2, 1)          # (MSH, S, N)
            m[f"w{i}"] = np.ascontiguousarray(wsb).reshape(MSH, NS).astype(np.float16)
        in_maps.append(m)
    return in_maps


def _run_shards(in_maps):
    from concourse.bass_utils import run_bass_kernel_spmd

    nc = _get_nc()
    return run_bass_kernel_spmd(nc, in_maps, list(range(NCORES)))


def kernel(x, wr, wg, wb, wc):
    res = _run_shards(_make_in_maps(x, wr, wg, wb, wc))
    X = np.empty((B, M, N, L), dtype=np.float32)
    for core in range(NCORES):
        b, mh = divmod(core, 2)
        xo = res.results[core]["out"].astype(np.float32).reshape(MSH, L, N).transpose(0, 2, 1)
        X[b, mh * MSH : (mh + 1) * MSH] = xo
    return X / X.max()


def estimate_ns() -> float:
    """Single-core cost-model estimate of the kernel duration (ns)."""
    from concourse.timeline_sim import TimelineSim

    return TimelineSim(_get_nc()).simulate()

